# revision 20
# baseline (speedup 1.0000x reference)
"""Cross-attention kernel for 8 TRN2 NeuronCores.

Reference computation (per batch b, c=1024 tokens, dim=1024):
    q = xf @ Wq.T ; k,v = cf @ Wkv.T split
    out = softmax(q @ k.T / 32) @ v

Key algebraic restructure: scores = q @ k.T = x Wq^T Wk c^T, so fold the
two projection weights into M'[e,d] = sum_o Wk[o,e] Wq[o,d] ONCE per core
(batch-independent, amortized over both local batches), then per batch a
single matmul tT[d,j] = sum_e M'[e,d] cT[e,j] replaces BOTH the q and k
projections.  Per-core PE work drops from 10 to 9 units of 2*1024^3 flops.

Sharding: data-parallel over batch (16 batches -> 2 per core), SPMD on 8
cores, no collectives.  All activations enter the device pre-transposed
(host-side) so every matmul has its contraction dim on SBUF partitions:

    M'[e,d]  = wk.T @ wq          (lhsT=wk[o,e], rhs=wq[o,d], natural layouts)
    tT[d,j]  = M'.T @ cT          (lhsT=M'[e,d], rhs=cT[e,j])
    v[j,o]   = cT.T  @ wvT        (lhsT=cT[e,j], rhs=wvT[e,o])
    ST[j,i]  = tT.T  @ xT         (scores, transposed; lhsT=tT[d,j], rhs=xT[d,i])
    ET       = exp(ST/32)         (ACT, scale fused; no max-subtraction --
                                   logits are ~N(0,1), exp is fp32-safe)
    out'[i,o] = ET.T @ v          (lhsT=ET[j,i], rhs=v[j,o])
    l[i]      = ET.T @ ones       (same stationary weights as out')
    out[i,o]  = out' * (1/l)      (DVE per-partition scale on PSUM->SBUF copy)

The M' phase is emitted in waves of 3 column-tiles with interleaved
accumulation over the o-subtiles so the PE consumes weight chunks at the
rate the startup DMA delivers them (no long head stall).  Output is DMA'd
in fp16 (upcast on host) to halve the write traffic and the final drain.
"""

import os
import sys

import numpy as np


def _ensure_paths():
    for p in ("/opt/trn_rl_repo", "/root/.axon_site/_ro/trn_rl_repo"):
        if os.path.isdir(p) and p not in sys.path:
            sys.path.append(p)


try:
    import concourse.bass  # noqa: F401
except ImportError:
    _ensure_paths()

import concourse.bass as bass  # noqa: E402
import concourse.tile as tile  # noqa: E402
from concourse import bacc, mybir  # noqa: E402
from concourse import bass_utils  # noqa: E402

B, C, HH, WW = 16, 1024, 32, 32
D = HH * WW  # 1024
NCORES = 8
BPC = B // NCORES  # 2 batches per core
P = 128
KS = D // P  # 8 contraction subtiles
NT = C // P  # 8 row tiles
NH = 512  # matmul moving free dim (one PSUM bank)
SCALE = float(D) ** -0.5

CDT = mybir.dt.float16  # on-device compute dtype
NPDT = np.float16

F32 = mybir.dt.float32

WARMUP_MMS = int(os.environ.get("KERNEL_WARMUP_MMS", "20"))


def _emit(tc, xT, cT, wqn, wkn, wv, out):
    nc = tc.nc
    from contextlib import ExitStack

    ctx = ExitStack()
    with ctx:
        wpool = ctx.enter_context(tc.tile_pool(name="weights", bufs=1))
        iopool = ctx.enter_context(tc.tile_pool(name="io", bufs=2))
        actpool = ctx.enter_context(tc.tile_pool(name="acts", bufs=1))
        outpool = ctx.enter_context(tc.tile_pool(name="outs", bufs=3))
        smpool = ctx.enter_context(tc.tile_pool(name="small", bufs=2))
        psum = ctx.enter_context(tc.tile_pool(name="psum", bufs=6, space="PSUM"))
        psuml = ctx.enter_context(tc.tile_pool(name="psuml", bufs=2, space="PSUM"))

        # Pre-warm the PE during the startup DMA window: HAM un-throttles
        # (1.2 -> 2.4 GHz) only after ~3.4us of sustained PE activity, so a
        # burst of throwaway matmuls here means the real stream starts warm.
        if WARMUP_MMS:
            # No memset: every engine is blocked on the framework start
            # barrier until ~3.2us, and a DVE memset would further chain the
            # warmup MMs behind the DVE table-load preamble (~5us) -- by then
            # the real weight chunks have landed and the warmup only delays
            # them.  Instead read a RAW (non-tile-tracked) SBUF scratch so
            # the MMs have no dependencies at all and issue the moment the
            # PE clears the barrier; multiplying garbage is fine since the
            # product is discarded.
            warm_in = ctx.enter_context(
                nc.sbuf_tensor("warm_in", [P, 128], CDT)
            ).ap()
            warm_ps = psum.tile([P, 128], F32, tag="mm", name="warm_ps")
            for _ in range(WARMUP_MMS):
                nc.tensor.matmul(
                    warm_ps[:],
                    lhsT=warm_in,
                    rhs=warm_in,
                    start=True,
                    stop=True,
                )

        # Weights resident for the whole kernel; inputs for both batches
        # prefetched up front.  DMA issue order is chosen so the bytes the
        # PE needs first land first: the M' phase needs wk+wq (interleaved
        # per o-subtile so accumulation can start after the first chunk),
        # then wv + batch-0 c (v/tT phases), then batch-0 x, then batch 1.
        wk_sb = wpool.tile([P, KS, D], CDT, tag="wk", name="wk_sb")
        wq_sb = wpool.tile([P, KS, D], CDT, tag="wq", name="wq_sb")
        wv_sb = wpool.tile([P, KS, D], CDT, tag="wv", name="wv_sb")
        m_sb = wpool.tile([P, KS, D], CDT, tag="m", name="m_sb")
        x_sbs = [
            iopool.tile([P, KS, C], CDT, tag="x", name="x_sb") for _ in range(BPC)
        ]
        c_sbs = [
            iopool.tile([P, KS, C], CDT, tag="c", name="c_sb") for _ in range(BPC)
        ]
        # Descriptor issue is ~650ns each and serial per engine queue; split
        # the startup stream across the two hardware-DGE queues (Sync and
        # Scalar) and send the h0 HALF of every weight chunk first: M' wave 1
        # runs h-outer, so its first ~7us of matmuls only need these 2MB --
        # halving the data the head waits on.
        qs = (nc.sync, nc.scalar)
        qi = 0
        for h in range(2):
            for o in range(KS):
                eng = qs[qi % 2]
                qi += 1
                eng.dma_start(
                    wk_sb[:, o, h * NH : (h + 1) * NH], wkn[o, :, h * NH : (h + 1) * NH]
                )
                eng.dma_start(
                    wq_sb[:, o, h * NH : (h + 1) * NH], wqn[o, :, h * NH : (h + 1) * NH]
                )
        # c0 fully before wv: phase A (tT) consumes c0 immediately after M'
        # closes, while v only needs wv a full phase later.
        for e in range(KS):
            nc.scalar.dma_start(c_sbs[0][:, e, :], cT[0, e])
        for e in range(KS):
            nc.scalar.dma_start(wv_sb[:, e, :], wv[e])
        for ks in range(KS):
            nc.scalar.dma_start(x_sbs[0][:, ks, :], xT[0, ks])
        for n in range(1, BPC):
            for ks in range(KS):
                nc.sync.dma_start(c_sbs[n][:, ks, :], cT[n, ks])
                nc.sync.dma_start(x_sbs[n][:, ks, :], xT[n, ks])

        ones = wpool.tile([P, 1], CDT, tag="ones", name="ones")
        nc.vector.memset(ones[:], 1.0)

        # ---- phase M: M'[e,d] = wk.T @ wq, once per core ----
        # Emitted in waves of 4 e-tiles (all 8 PSUM banks -- the 4th group
        # borrows the phase-D l-pool's 2 banks, idle until then) with the
        # o-subtile accumulation loop OUTER, so wave 1's ~14us of matmuls
        # fully cover the startup weight-DMA window with no pacing idle.
        for wave in ([0, 1, 2, 3], [4, 5, 6, 7]):
            ps = {}
            for wi, e in enumerate(wave):
                pool = psuml if wi == 3 else psum
                tag = "l" if wi == 3 else "mm"
                ps[e] = [
                    pool.tile([P, NH], F32, tag=tag, name="ps_mm") for _ in range(2)
                ]
            # h OUTER: wave 1's e-tiles {0..3} have their stationaries in the
            # h0 half of wk, so the whole h0 sweep needs only the first-DMA'd
            # 2MB of weights; the h1 sweep starts after its halves land.
            for h in range(2):
                for o in range(KS):
                    for e in wave:
                        nc.tensor.matmul(
                            ps[e][h][:],
                            lhsT=wk_sb[:, o, e * P : (e + 1) * P],
                            rhs=wq_sb[:, o, h * NH : (h + 1) * NH],
                            start=(o == 0),
                            stop=(o == KS - 1),
                        )
            for e in wave:
                for h in range(2):
                    nc.vector.tensor_copy(
                        m_sb[:, e, h * NH : (h + 1) * NH], ps[e][h][:]
                    )

        for n in range(BPC):
            x_sb = x_sbs[n]
            c_sb = c_sbs[n]

            # ---- phase A: tT[d,j] = M'.T @ cT ----
            tT_sb = actpool.tile([P, KS, C], CDT, tag="tT", name="tT_sb")
            for dt in range(KS):
                ps = [psum.tile([P, NH], F32, tag="mm", name="ps_mm") for _ in range(2)]
                for e in range(KS):
                    for h in range(2):
                        nc.tensor.matmul(
                            ps[h][:],
                            lhsT=m_sb[:, e, dt * P : (dt + 1) * P],
                            rhs=c_sb[:, e, h * NH : (h + 1) * NH],
                            start=(e == 0),
                            stop=(e == KS - 1),
                        )
                for h in range(2):
                    nc.vector.tensor_copy(
                        tT_sb[:, dt, h * NH : (h + 1) * NH], ps[h][:]
                    )

            # ---- phase B: v[j,o] = cT.T @ wvT ----
            v_sb = actpool.tile([P, KS, D], CDT, tag="v", name="v_sb")
            for jt in range(NT):
                ps = [psum.tile([P, NH], F32, tag="mm", name="ps_mm") for _ in range(2)]
                for e in range(KS):
                    for h in range(2):
                        nc.tensor.matmul(
                            ps[h][:],
                            lhsT=c_sb[:, e, jt * P : (jt + 1) * P],
                            rhs=wv_sb[:, e, h * NH : (h + 1) * NH],
                            start=(e == 0),
                            stop=(e == KS - 1),
                        )
                for h in range(2):
                    nc.vector.tensor_copy(
                        v_sb[:, jt, h * NH : (h + 1) * NH], ps[h][:]
                    )

            # ---- phase C: ST[j,i] = tT.T @ xT -> ET = exp(ST/32) ----
            eT_sb = actpool.tile([P, KS, C], CDT, tag="eT", name="eT_sb")
            for jt in range(NT):
                ps = [psum.tile([P, NH], F32, tag="mm", name="ps_mm") for _ in range(2)]
                for ds in range(KS):
                    for h in range(2):
                        nc.tensor.matmul(
                            ps[h][:],
                            lhsT=tT_sb[:, ds, jt * P : (jt + 1) * P],
                            rhs=x_sb[:, ds, h * NH : (h + 1) * NH],
                            start=(ds == 0),
                            stop=(ds == KS - 1),
                        )
                for h in range(2):
                    nc.scalar.activation(
                        eT_sb[:, jt, h * NH : (h + 1) * NH],
                        ps[h][:],
                        mybir.ActivationFunctionType.Exp,
                        scale=SCALE,
                    )

            # ---- phase D: out'[i,o] = ET.T @ v ; l = ET.T @ ones ; scale ----
            for it in range(NT):
                o_sb = outpool.tile([P, D], CDT, tag="o", name="o_sb")
                ps = [psum.tile([P, NH], F32, tag="mm", name="ps_mm") for _ in range(2)]
                psl = psuml.tile([P, 1], F32, tag="l", name="ps_l")
                for js in range(NT):
                    lhsT = eT_sb[:, js, it * P : (it + 1) * P]
                    for h in range(2):
                        nc.tensor.matmul(
                            ps[h][:],
                            lhsT=lhsT,
                            rhs=v_sb[:, js, h * NH : (h + 1) * NH],
                            start=(js == 0),
                            stop=(js == NT - 1),
                        )
                    nc.tensor.matmul(
                        psl[:],
                        lhsT=lhsT,
                        rhs=ones[:, 0:1],
                        start=(js == 0),
                        stop=(js == NT - 1),
                    )
                r_it = smpool.tile([P, 1], F32, tag="r", name="r_it")
                nc.vector.reciprocal(r_it[:], psl[:])
                for h in range(2):
                    nc.vector.tensor_scalar_mul(
                        o_sb[:, h * NH : (h + 1) * NH], ps[h][:], r_it[:]
                    )
                    # alternate output queues so the final two descriptors
                    # process in parallel instead of back-to-back
                    eng = nc.sync if h == 0 else nc.scalar
                    eng.dma_start(
                        out[n, it, :, h * NH : (h + 1) * NH],
                        o_sb[:, h * NH : (h + 1) * NH],
                    )


_NC_CACHE = {}


def _build():
    if "nc" in _NC_CACHE:
        return _NC_CACHE["nc"]
    nc = bacc.Bacc("TRN2", target_bir_lowering=False, debug=False)
    xT = nc.dram_tensor("xT", [BPC, KS, P, C], CDT, kind="ExternalInput").ap()
    cT = nc.dram_tensor("cT", [BPC, KS, P, C], CDT, kind="ExternalInput").ap()
    wqn = nc.dram_tensor("wqn", [KS, P, D], CDT, kind="ExternalInput").ap()
    wkn = nc.dram_tensor("wkn", [KS, P, D], CDT, kind="ExternalInput").ap()
    wv = nc.dram_tensor("wv", [KS, P, D], CDT, kind="ExternalInput").ap()
    out = nc.dram_tensor("out", [BPC, NT, P, D], CDT, kind="ExternalOutput").ap()
    with tile.TileContext(nc) as tc:
        _emit(tc, xT, cT, wqn, wkn, wv, out)
    nc.compile()
    _NC_CACHE["nc"] = nc
    return nc


def kernel(**inputs) -> np.ndarray:
    x = np.asarray(inputs["x"], dtype=np.float32).reshape(B, C, D)
    cond = np.asarray(inputs["cond_img"], dtype=np.float32).reshape(B, C, D)
    Wq = np.asarray(inputs["Wq"], dtype=np.float32)
    Wkv = np.asarray(inputs["Wkv"], dtype=np.float32)

    # Pre-transpose activations on host so the contraction dim lands on
    # partitions.  wq/wk ship in NATURAL (out_features-major) layout -- the
    # M' fold contracts over the OUTPUT feature axis o.
    xT = np.ascontiguousarray(x.transpose(0, 2, 1)).astype(NPDT)  # (B, D, C)
    cT = np.ascontiguousarray(cond.transpose(0, 2, 1)).astype(NPDT)
    wqn = Wq.astype(NPDT)  # (o, d) natural
    wkn = Wkv[:D].astype(NPDT)  # (o, e) natural
    wvT = np.ascontiguousarray(Wkv[D:].T).astype(NPDT)  # (e, o)

    xT = xT.reshape(NCORES, BPC, KS, P, C)
    cT = cT.reshape(NCORES, BPC, KS, P, C)
    wqn = wqn.reshape(KS, P, D)
    wkn = wkn.reshape(KS, P, D)
    wv = wvT.reshape(KS, P, D)

    in_maps = [
        {"xT": xT[i], "cT": cT[i], "wqn": wqn, "wkn": wkn, "wv": wv}
        for i in range(NCORES)
    ]

    nc = _build()
    trace = bool(os.environ.get("KERNEL_TRACE"))
    res = bass_utils.run_bass_kernel_spmd(
        nc, in_maps, core_ids=list(range(NCORES)), trace=trace
    )
    if trace:
        _NC_CACHE["last_result"] = res

    outs = np.stack([np.asarray(res.results[i]["out"]) for i in range(NCORES)])
    return outs.reshape(B, C, HH, WW).astype(np.float32)


# revision 21
# speedup vs baseline: 1.0038x; 1.0038x over previous
"""Cross-attention kernel for 8 TRN2 NeuronCores.

Reference computation (per batch b, c=1024 tokens, dim=1024):
    q = xf @ Wq.T ; k,v = cf @ Wkv.T split
    out = softmax(q @ k.T / 32) @ v

Key algebraic restructure: scores = q @ k.T = x Wq^T Wk c^T, so fold the
two projection weights into M'[e,d] = sum_o Wk[o,e] Wq[o,d] ONCE per core
(batch-independent, amortized over both local batches), then per batch a
single matmul tT[d,j] = sum_e M'[e,d] cT[e,j] replaces BOTH the q and k
projections.  Per-core PE work drops from 10 to 9 units of 2*1024^3 flops.

Sharding: data-parallel over batch (16 batches -> 2 per core), SPMD on 8
cores, no collectives.  All activations enter the device pre-transposed
(host-side) so every matmul has its contraction dim on SBUF partitions:

    M'[e,d]  = wk.T @ wq          (lhsT=wk[o,e], rhs=wq[o,d], natural layouts)
    tT[d,j]  = M'.T @ cT          (lhsT=M'[e,d], rhs=cT[e,j])
    v[j,o]   = cT.T  @ wvT        (lhsT=cT[e,j], rhs=wvT[e,o])
    ST[j,i]  = tT.T  @ xT         (scores, transposed; lhsT=tT[d,j], rhs=xT[d,i])
    ET       = exp(ST/32)         (ACT, scale fused; no max-subtraction --
                                   logits are ~N(0,1), exp is fp32-safe)
    out'[i,o] = ET.T @ v          (lhsT=ET[j,i], rhs=v[j,o])
    l[i]      = ET.T @ ones       (same stationary weights as out')
    out[i,o]  = out' * (1/l)      (DVE per-partition scale on PSUM->SBUF copy)

The M' phase is emitted in waves of 3 column-tiles with interleaved
accumulation over the o-subtiles so the PE consumes weight chunks at the
rate the startup DMA delivers them (no long head stall).  Output is DMA'd
in fp16 (upcast on host) to halve the write traffic and the final drain.
"""

import os
import sys

import numpy as np


def _ensure_paths():
    for p in ("/opt/trn_rl_repo", "/root/.axon_site/_ro/trn_rl_repo"):
        if os.path.isdir(p) and p not in sys.path:
            sys.path.append(p)


try:
    import concourse.bass  # noqa: F401
except ImportError:
    _ensure_paths()

import concourse.bass as bass  # noqa: E402
import concourse.tile as tile  # noqa: E402
from concourse import bacc, mybir  # noqa: E402
from concourse import bass_utils  # noqa: E402

B, C, HH, WW = 16, 1024, 32, 32
D = HH * WW  # 1024
NCORES = 8
BPC = B // NCORES  # 2 batches per core
P = 128
KS = D // P  # 8 contraction subtiles
NT = C // P  # 8 row tiles
NH = 512  # matmul moving free dim (one PSUM bank)
SCALE = float(D) ** -0.5

CDT = mybir.dt.float16  # on-device compute dtype
NPDT = np.float16

F32 = mybir.dt.float32

WARMUP_MMS = int(os.environ.get("KERNEL_WARMUP_MMS", "20"))


def _emit(tc, xT, cT, wqn, wkn, wv, out):
    nc = tc.nc
    from contextlib import ExitStack

    ctx = ExitStack()
    with ctx:
        wpool = ctx.enter_context(tc.tile_pool(name="weights", bufs=1))
        iopool = ctx.enter_context(tc.tile_pool(name="io", bufs=2))
        actpool = ctx.enter_context(tc.tile_pool(name="acts", bufs=1))
        outpool = ctx.enter_context(tc.tile_pool(name="outs", bufs=3))
        smpool = ctx.enter_context(tc.tile_pool(name="small", bufs=2))
        psum = ctx.enter_context(tc.tile_pool(name="psum", bufs=6, space="PSUM"))
        psuml = ctx.enter_context(tc.tile_pool(name="psuml", bufs=2, space="PSUM"))

        # Pre-warm the PE during the startup DMA window: HAM un-throttles
        # (1.2 -> 2.4 GHz) only after ~3.4us of sustained PE activity, so a
        # burst of throwaway matmuls here means the real stream starts warm.
        if WARMUP_MMS:
            # No memset: every engine is blocked on the framework start
            # barrier until ~3.2us, and a DVE memset would further chain the
            # warmup MMs behind the DVE table-load preamble (~5us) -- by then
            # the real weight chunks have landed and the warmup only delays
            # them.  Instead read a RAW (non-tile-tracked) SBUF scratch so
            # the MMs have no dependencies at all and issue the moment the
            # PE clears the barrier; multiplying garbage is fine since the
            # product is discarded.
            warm_in = ctx.enter_context(
                nc.sbuf_tensor("warm_in", [P, 128], CDT)
            ).ap()
            warm_ps = psum.tile([P, 128], F32, tag="mm", name="warm_ps")
            for _ in range(WARMUP_MMS):
                nc.tensor.matmul(
                    warm_ps[:],
                    lhsT=warm_in,
                    rhs=warm_in,
                    start=True,
                    stop=True,
                )

        # Weights resident for the whole kernel; inputs for both batches
        # prefetched up front.  DMA issue order is chosen so the bytes the
        # PE needs first land first: the M' phase needs wk+wq (interleaved
        # per o-subtile so accumulation can start after the first chunk),
        # then wv + batch-0 c (v/tT phases), then batch-0 x, then batch 1.
        wk_sb = wpool.tile([P, KS, D], CDT, tag="wk", name="wk_sb")
        wq_sb = wpool.tile([P, KS, D], CDT, tag="wq", name="wq_sb")
        wv_sb = wpool.tile([P, KS, D], CDT, tag="wv", name="wv_sb")
        m_sb = wpool.tile([P, KS, D], CDT, tag="m", name="m_sb")
        x_sbs = [
            iopool.tile([P, KS, C], CDT, tag="x", name="x_sb") for _ in range(BPC)
        ]
        c_sbs = [
            iopool.tile([P, KS, C], CDT, tag="c", name="c_sb") for _ in range(BPC)
        ]
        # Descriptor issue is ~650ns each and serial per engine queue; split
        # the startup stream across the Sync AND Scalar queues (the scalar
        # engine is idle until the first exp at ~90us) so the weight chunks
        # that pace M' wave 1 start flowing in half the time.
        for o in range(KS):
            eng = nc.sync if o % 2 == 0 else nc.scalar
            eng.dma_start(wk_sb[:, o, :], wkn[o])
            eng.dma_start(wq_sb[:, o, :], wqn[o])
        # c0 fully before wv: phase A (tT) consumes c0 immediately after M'
        # closes, while v only needs wv a full phase later.
        for e in range(KS):
            nc.scalar.dma_start(c_sbs[0][:, e, :], cT[0, e])
        for e in range(KS):
            nc.scalar.dma_start(wv_sb[:, e, :], wv[e])
        for ks in range(KS):
            nc.scalar.dma_start(x_sbs[0][:, ks, :], xT[0, ks])
        for n in range(1, BPC):
            for ks in range(KS):
                nc.sync.dma_start(c_sbs[n][:, ks, :], cT[n, ks])
                nc.sync.dma_start(x_sbs[n][:, ks, :], xT[n, ks])

        ones = wpool.tile([P, 1], CDT, tag="ones", name="ones")
        nc.vector.memset(ones[:], 1.0)

        # ---- phase M: M'[e,d] = wk.T @ wq, once per core ----
        # Emitted in waves of 4 e-tiles (all 8 PSUM banks -- the 4th group
        # borrows the phase-D l-pool's 2 banks, idle until then) with the
        # o-subtile accumulation loop OUTER, so wave 1's ~14us of matmuls
        # fully cover the startup weight-DMA window with no pacing idle.
        for wave in ([0, 1, 2, 3], [4, 5, 6, 7]):
            ps = {}
            for wi, e in enumerate(wave):
                pool = psuml if wi == 3 else psum
                tag = "l" if wi == 3 else "mm"
                ps[e] = [
                    pool.tile([P, NH], F32, tag=tag, name="ps_mm") for _ in range(2)
                ]
            for o in range(KS):
                for e in wave:
                    for h in range(2):
                        nc.tensor.matmul(
                            ps[e][h][:],
                            lhsT=wk_sb[:, o, e * P : (e + 1) * P],
                            rhs=wq_sb[:, o, h * NH : (h + 1) * NH],
                            start=(o == 0),
                            stop=(o == KS - 1),
                        )
            for e in wave:
                for h in range(2):
                    nc.vector.tensor_copy(
                        m_sb[:, e, h * NH : (h + 1) * NH], ps[e][h][:]
                    )

        for n in range(BPC):
            x_sb = x_sbs[n]
            c_sb = c_sbs[n]

            # ---- phase A: tT[d,j] = M'.T @ cT ----
            tT_sb = actpool.tile([P, KS, C], CDT, tag="tT", name="tT_sb")
            for dt in range(KS):
                ps = [psum.tile([P, NH], F32, tag="mm", name="ps_mm") for _ in range(2)]
                for e in range(KS):
                    for h in range(2):
                        nc.tensor.matmul(
                            ps[h][:],
                            lhsT=m_sb[:, e, dt * P : (dt + 1) * P],
                            rhs=c_sb[:, e, h * NH : (h + 1) * NH],
                            start=(e == 0),
                            stop=(e == KS - 1),
                        )
                for h in range(2):
                    nc.vector.tensor_copy(
                        tT_sb[:, dt, h * NH : (h + 1) * NH], ps[h][:]
                    )

            # ---- phase B: v[j,o] = cT.T @ wvT ----
            v_sb = actpool.tile([P, KS, D], CDT, tag="v", name="v_sb")
            for jt in range(NT):
                ps = [psum.tile([P, NH], F32, tag="mm", name="ps_mm") for _ in range(2)]
                for e in range(KS):
                    for h in range(2):
                        nc.tensor.matmul(
                            ps[h][:],
                            lhsT=c_sb[:, e, jt * P : (jt + 1) * P],
                            rhs=wv_sb[:, e, h * NH : (h + 1) * NH],
                            start=(e == 0),
                            stop=(e == KS - 1),
                        )
                for h in range(2):
                    nc.vector.tensor_copy(
                        v_sb[:, jt, h * NH : (h + 1) * NH], ps[h][:]
                    )

            # ---- phase C: ST[j,i] = tT.T @ xT -> ET = exp(ST/32) ----
            eT_sb = actpool.tile([P, KS, C], CDT, tag="eT", name="eT_sb")
            for jt in range(NT):
                ps = [psum.tile([P, NH], F32, tag="mm", name="ps_mm") for _ in range(2)]
                for ds in range(KS):
                    for h in range(2):
                        nc.tensor.matmul(
                            ps[h][:],
                            lhsT=tT_sb[:, ds, jt * P : (jt + 1) * P],
                            rhs=x_sb[:, ds, h * NH : (h + 1) * NH],
                            start=(ds == 0),
                            stop=(ds == KS - 1),
                        )
                for h in range(2):
                    nc.scalar.activation(
                        eT_sb[:, jt, h * NH : (h + 1) * NH],
                        ps[h][:],
                        mybir.ActivationFunctionType.Exp,
                        scale=SCALE,
                    )

            # ---- phase D: out'[i,o] = ET.T @ v ; l = ET.T @ ones ; scale ----
            for it in range(NT):
                o_sb = outpool.tile([P, D], CDT, tag="o", name="o_sb")
                ps = [psum.tile([P, NH], F32, tag="mm", name="ps_mm") for _ in range(2)]
                psl = psuml.tile([P, 1], F32, tag="l", name="ps_l")
                for js in range(NT):
                    lhsT = eT_sb[:, js, it * P : (it + 1) * P]
                    for h in range(2):
                        nc.tensor.matmul(
                            ps[h][:],
                            lhsT=lhsT,
                            rhs=v_sb[:, js, h * NH : (h + 1) * NH],
                            start=(js == 0),
                            stop=(js == NT - 1),
                        )
                    nc.tensor.matmul(
                        psl[:],
                        lhsT=lhsT,
                        rhs=ones[:, 0:1],
                        start=(js == 0),
                        stop=(js == NT - 1),
                    )
                r_it = smpool.tile([P, 1], F32, tag="r", name="r_it")
                nc.vector.reciprocal(r_it[:], psl[:])
                for h in range(2):
                    nc.vector.tensor_scalar_mul(
                        o_sb[:, h * NH : (h + 1) * NH], ps[h][:], r_it[:]
                    )
                    # alternate output queues so the final two descriptors
                    # process in parallel instead of back-to-back
                    eng = nc.sync if h == 0 else nc.scalar
                    eng.dma_start(
                        out[n, it, :, h * NH : (h + 1) * NH],
                        o_sb[:, h * NH : (h + 1) * NH],
                    )


_NC_CACHE = {}


def _build():
    if "nc" in _NC_CACHE:
        return _NC_CACHE["nc"]
    nc = bacc.Bacc("TRN2", target_bir_lowering=False, debug=False)
    xT = nc.dram_tensor("xT", [BPC, KS, P, C], CDT, kind="ExternalInput").ap()
    cT = nc.dram_tensor("cT", [BPC, KS, P, C], CDT, kind="ExternalInput").ap()
    wqn = nc.dram_tensor("wqn", [KS, P, D], CDT, kind="ExternalInput").ap()
    wkn = nc.dram_tensor("wkn", [KS, P, D], CDT, kind="ExternalInput").ap()
    wv = nc.dram_tensor("wv", [KS, P, D], CDT, kind="ExternalInput").ap()
    out = nc.dram_tensor("out", [BPC, NT, P, D], CDT, kind="ExternalOutput").ap()
    with tile.TileContext(nc) as tc:
        _emit(tc, xT, cT, wqn, wkn, wv, out)
    nc.compile()
    _NC_CACHE["nc"] = nc
    return nc


def kernel(**inputs) -> np.ndarray:
    x = np.asarray(inputs["x"], dtype=np.float32).reshape(B, C, D)
    cond = np.asarray(inputs["cond_img"], dtype=np.float32).reshape(B, C, D)
    Wq = np.asarray(inputs["Wq"], dtype=np.float32)
    Wkv = np.asarray(inputs["Wkv"], dtype=np.float32)

    # Pre-transpose activations on host so the contraction dim lands on
    # partitions.  wq/wk ship in NATURAL (out_features-major) layout -- the
    # M' fold contracts over the OUTPUT feature axis o.
    xT = np.ascontiguousarray(x.transpose(0, 2, 1)).astype(NPDT)  # (B, D, C)
    cT = np.ascontiguousarray(cond.transpose(0, 2, 1)).astype(NPDT)
    wqn = Wq.astype(NPDT)  # (o, d) natural
    wkn = Wkv[:D].astype(NPDT)  # (o, e) natural
    wvT = np.ascontiguousarray(Wkv[D:].T).astype(NPDT)  # (e, o)

    xT = xT.reshape(NCORES, BPC, KS, P, C)
    cT = cT.reshape(NCORES, BPC, KS, P, C)
    wqn = wqn.reshape(KS, P, D)
    wkn = wkn.reshape(KS, P, D)
    wv = wvT.reshape(KS, P, D)

    in_maps = [
        {"xT": xT[i], "cT": cT[i], "wqn": wqn, "wkn": wkn, "wv": wv}
        for i in range(NCORES)
    ]

    nc = _build()
    trace = bool(os.environ.get("KERNEL_TRACE"))
    res = bass_utils.run_bass_kernel_spmd(
        nc, in_maps, core_ids=list(range(NCORES)), trace=trace
    )
    if trace:
        _NC_CACHE["last_result"] = res

    outs = np.stack([np.asarray(res.results[i]["out"]) for i in range(NCORES)])
    return outs.reshape(B, C, HH, WW).astype(np.float32)
